# revision 1
# baseline (speedup 1.0000x reference)
"""Trainium2 Bass kernel for nn_DCMCLITA (conv + BiLSTM siamese geo model).

Strategy:
  - Host (numpy): faithful preprocessing (haversine speed injection, mercator
    normalize), conv1d feature build, input projection xg = feat @ W_ih.T + b,
    the trivial backward-direction single cells (reference's reverse-scan
    output at index -1 only sees the last timestep), the tiny x3 branch
    (L=2), and the FC head.
  - Device (8 NeuronCores, Bass/Tile): the two heavy 512-step forward LSTM
    recurrences (x1 & x2 branches share forward weights) -> data parallel:
    each core runs ONE merged 32-row chain (16 samples x 2 branches).

Per-step device math (gate-dim on partitions, rows on free dim):
    z = I.T @ xg_t  (+)  W_k0.T @ h[0:128]  (+)  W_k1.T @ h[128:256]   (PSUM)
    A_ifo = sigmoid(z[ifo]); tg = tanh(z[g])                            (ACT)
    u = A_i*tg ; v = A_f*c ; c' = u+v (fp32) ; T = tanh(c') ; h = A_o*T
"""

import os
import numpy as np

B, L, C, H = 128, 512, 6, 256
NCORES = 8
SPC = B // NCORES          # samples per core
ROWS = 2 * SPC             # 32 rows per core-chain (x1 + x2 branches)
CHUNK = 64                 # timesteps per DMA chunk
R_MERC = 6378137.0
R_EARTH = 6371.0

_sig = lambda x: 1.0 / (1.0 + np.exp(-x))


def _conv_feat(x, p):
    # x: (B, L, 6) float32 -> feat (B, L, 198) = [x, relu(c1), relu(c3), relu(c5)]
    outs = [x]
    for K, pad, wk, bk in ((1, 0, 'conv1_w', 'conv1_b'), (3, 1, 'conv3_w', 'conv3_b'),
                           (5, 2, 'conv5_w', 'conv5_b')):
        w, b = p[wk], p[bk]            # (64, 6, K), (64,)
        xp = np.pad(x, ((0, 0), (pad, pad), (0, 0)))
        acc = np.zeros((x.shape[0], x.shape[1], 64), np.float32)
        for j in range(K):
            acc += xp[:, j:j + x.shape[1], :] @ w[:, :, j].T
        outs.append(np.maximum(acc + b, 0.0))
    return np.concatenate(outs, axis=-1).astype(np.float32)


def _merc_x(lon):
    return R_MERC * np.deg2rad(lon)


def _merc_y(lat):
    return R_MERC * np.log(np.tan(np.pi / 4 + np.deg2rad(lat) / 2))


def _preprocess(x1, x2, dtime):
    x1 = x1.astype(np.float32).copy()
    x2 = x2.astype(np.float32).copy()
    lat1, lon1 = x1[:, -1, 0], x1[:, -1, 1]
    lat2, lon2 = x2[:, 0, 0], x2[:, 0, 1]
    la1, lo1, la2, lo2 = map(np.deg2rad, (lat1, lon1, lat2, lon2))
    dlon, dlat = lo2 - lo1, la2 - la1
    a = np.sin(dlat / 2) ** 2 + np.cos(la1) * np.cos(la2) * np.sin(dlon / 2) ** 2
    dist = 2.0 * np.arcsin(np.sqrt(a)) * R_EARTH
    yb = np.sin(dlon) * np.cos(la2)
    xb = np.cos(la1) * np.sin(la2) - np.sin(la1) * np.cos(la2) * np.cos(dlon)
    brg = np.deg2rad((np.degrees(np.arctan2(yb, xb)) + 360.0) % 360.0)
    dt = dtime.reshape(-1).astype(np.float32)
    dt = np.where(dt == 0, np.float32(1.0), dt)
    speeds = dist / dt * 1000.0 / 0.514444
    vx, vy = speeds * np.sin(brg), speeds * np.cos(brg)
    x2[:, 0, 2] = np.where(speeds != 0, speeds, x2[:, 0, 2])
    x2[:, 0, 4] = np.where(vx != 0, vx, x2[:, 0, 4])
    x2[:, 0, 5] = np.where(vy != 0, vy, x2[:, 0, 5])
    x3 = np.concatenate([x1[:, -1:, :], x2[:, 0:1, :]], axis=1)

    a1 = _merc_x(x1[:, :, 1]); b1 = _merc_y(x1[:, :, 0])
    a2 = _merc_x(x2[:, :, 1]); b2 = _merc_y(x2[:, :, 0])
    max_lat = np.maximum(a1.max(1, keepdims=True), a2.max(1, keepdims=True))
    min_lat = np.minimum(a1.min(1, keepdims=True), a2.min(1, keepdims=True))
    max_lon = np.maximum(b1.max(1, keepdims=True), b2.max(1, keepdims=True))
    min_lon = np.minimum(b1.min(1, keepdims=True), b2.min(1, keepdims=True))
    eps = np.float32(1e-8)
    dla = max_lat - min_lat + eps
    dlo = max_lon - min_lon + eps
    x1[:, :, 0] = (a1 - min_lat) / dla; x1[:, :, 1] = (b1 - min_lon) / dlo
    x2[:, :, 0] = (a2 - min_lat) / dla; x2[:, :, 1] = (b2 - min_lon) / dlo
    lat3 = _merc_y(x3[:, :, 0]); lon3 = _merc_x(x3[:, :, 1])
    x3[:, :, 0] = (lat3 - min_lat) / dla; x3[:, :, 1] = (lon3 - min_lon) / dlo
    return x1.astype(np.float32), x2.astype(np.float32), x3.astype(np.float32)


def _lstm_fwd_np(xg):
    # xg: (B, T, 4H) pre-activations in torch gate order i,f,g,o. Returns final h.
    n = xg.shape[0]
    h = np.zeros((n, H), np.float32)
    c = np.zeros((n, H), np.float32)
    return h, c, xg  # placeholder; real loop in _lstm_run


def _lstm_run(xg, w_hh):
    n, T, _ = xg.shape
    h = np.zeros((n, H), np.float32)
    c = np.zeros((n, H), np.float32)
    for t in range(T):
        g = xg[:, t] + h @ w_hh.T
        i, f, gg, o = np.split(g, 4, axis=-1)
        c = _sig(f) * c + _sig(i) * np.tanh(gg)
        h = _sig(o) * np.tanh(c)
    return h


def _bwd_cell(feat_last, w_ih, w_hh, b_ih, b_hh):
    # reference's hb[:, -1] == one LSTM cell applied to the LAST timestep, zero state
    z = feat_last @ w_ih.T + b_ih + b_hh
    i, f, g, o = np.split(z, 4, axis=-1)
    c = _sig(i) * np.tanh(g)
    return _sig(o) * np.tanh(c)


# ---------------------------------------------------------------------------
# Bass device program (built once, cached)
# ---------------------------------------------------------------------------
_CACHE = {}


def _build_bass():
    from contextlib import ExitStack
    import concourse.bass as bass
    import concourse.bacc as bacc
    import concourse.tile as tile
    from concourse import mybir

    nc = bacc.Bacc("TRN2")
    bf16 = mybir.dt.bfloat16
    f32 = mybir.dt.float32

    MT = 8  # gate m-tiles
    NCHUNK = L // CHUNK
    CC = CHUNK * ROWS            # xg cols per chunk per m-tile (t-major, row-minor)
    NW = CC // 512               # 512-col psum windows per m-tile per chunk
    NPAIR = MT * NW              # xg matmul-pairs per chunk
    feat_d = nc.dram_tensor("feat", [2, 128, L, ROWS], bf16, kind="ExternalInput")
    wih_d = nc.dram_tensor("wih", [128, 2, 1024], bf16, kind="ExternalInput")
    whh_d = nc.dram_tensor("whh", [128, 2, 1024], bf16, kind="ExternalInput")
    ident_d = nc.dram_tensor("ident", [128, 128], bf16, kind="ExternalInput")
    hout_d = nc.dram_tensor("hout", [128, 2 * ROWS], bf16, kind="ExternalOutput")

    AF = mybir.ActivationFunctionType
    G = ROWS  # cols per gate m-tile slice in the z psum packing
    with tile.TileContext(nc) as tc:
        with ExitStack() as ctx:
            singles = ctx.enter_context(tc.tile_pool(name="singles", bufs=1))
            featp = ctx.enter_context(tc.tile_pool(name="featp", bufs=2))
            xgp = ctx.enter_context(tc.tile_pool(name="xgp", bufs=2))
            psums = ctx.enter_context(tc.tile_pool(name="ps", bufs=2, space="PSUM"))
            psxg = ctx.enter_context(tc.tile_pool(name="psxg", bufs=3, space="PSUM"))
            work = ctx.enter_context(tc.tile_pool(name="work", bufs=6))

            wih_s = singles.tile([128, 2, 1024], bf16)
            nc.sync.dma_start(out=wih_s, in_=wih_d[:])
            whh_s = singles.tile([128, 2, 1024], bf16)
            nc.sync.dma_start(out=whh_s, in_=whh_d[:])
            ident_s = singles.tile([128, 128], bf16)
            nc.sync.dma_start(out=ident_s, in_=ident_d[:])

            # Two independent 16-row chains (x1 rows / x2 rows), staggered so
            # one chain's gate tail overlaps the other chain's PE phase.
            GH = G // 2  # rows per chain
            h0_s = singles.tile([128, 2 * GH], bf16, tag="h0")
            h1_s = singles.tile([128, 2 * GH], bf16, tag="h1")
            h_s = [h0_s, h1_s]
            # S = [tanh(g) | c] side by side so one TT computes A_i*tg and A_f*c
            S0_s = singles.tile([128, 4 * GH], bf16, tag="S0")
            S1_s = singles.tile([128, 4 * GH], bf16, tag="S1")
            S_s = [S0_s, S1_s]
            for c in (0, 1):
                nc.vector.memset(h_s[c], 0.0)
                nc.vector.memset(S_s[c], 0.0)

            def dma_feat(chunk):
                t0 = chunk * CHUNK
                ft = featp.tile([128, 2, CC], bf16)
                nc.sync.dma_start(
                    out=ft,
                    in_=feat_d[:, :, t0:t0 + CHUNK, :].rearrange(
                        "k p c r -> p k (c r)"),
                )
                return ft

            def xg_mms(ft, i):
                # one [128,512] window of next chunk's xg: 2 MMs into PSUM
                m, nb = i // NW, i % NW
                ps = psxg.tile([128, 512], f32)
                nc.tensor.matmul(ps, wih_s[:, 0, m * 128:(m + 1) * 128],
                                 ft[:, 0, nb * 512:(nb + 1) * 512],
                                 start=True, stop=False)
                nc.tensor.matmul(ps, wih_s[:, 1, m * 128:(m + 1) * 128],
                                 ft[:, 1, nb * 512:(nb + 1) * 512],
                                 start=False, stop=True)
                return ps, m, nb

            def xg_evac(xg_s, pend):
                ps, m, nb = pend
                nc.scalar.copy(xg_s[:, m, nb * 512:(nb + 1) * 512], ps)

            def xg_pair(ft, xg_s, i):
                xg_evac(xg_s, xg_mms(ft, i))

            ft_cur = dma_feat(0)
            xg_cur = xgp.tile([128, MT, CC], bf16, tag="xg")
            for i in range(NPAIR):
                xg_pair(ft_cur, xg_cur, i)

            for chunk in range(NCHUNK):
                if chunk + 1 < NCHUNK:
                    ft_nxt = dma_feat(chunk + 1)
                    xg_nxt = xgp.tile([128, MT, CC], bf16, tag="xg")
                xg3 = xg_cur.rearrange("p m (c r) -> p m c r", r=ROWS)

                def chain_step(ch, tl):
                    rlo = ch * GH
                    z = psums.tile([128, MT * GH], f32, tag=f"z{ch}")
                    # single identity MM seeds the whole z bank with xg_t
                    nc.tensor.matmul(z, ident_s, xg3[:, :, tl, rlo:rlo + GH],
                                     start=True, stop=False,
                                     skip_group_check=True)
                    for m in range(MT):
                        zslice = z[:, m * GH:(m + 1) * GH]
                        nc.tensor.matmul(zslice, whh_s[:, 0, m * 128:(m + 1) * 128],
                                         h_s[ch][:, 0:GH], start=False, stop=False,
                                         skip_group_check=True)
                        nc.tensor.matmul(zslice, whh_s[:, 1, m * 128:(m + 1) * 128],
                                         h_s[ch][:, GH:2 * GH], start=False,
                                         stop=(m == MT - 1),
                                         skip_group_check=True)
                    # gate m-tile order: [i0,i1,f0,f1,o0,o1,g0,g1]; g pre-scaled
                    # x2 host-side so sigmoid(z_g) == sigmoid(2g) and
                    # tanh(g) = 2*sigmoid(2g) - 1  -> single sigmoid ACT op.
                    A = work.tile([128, 8 * GH], bf16, tag=f"A{ch}")
                    nc.scalar.activation(A, z, AF.Sigmoid)
                    nc.vector.tensor_scalar(S_s[ch][:, 0:2 * GH],
                                            A[:, 6 * GH:8 * GH],
                                            2.0, -1.0, mybir.AluOpType.mult,
                                            mybir.AluOpType.add)
                    uv = work.tile([128, 4 * GH], bf16, tag=f"uv{ch}")
                    nc.vector.tensor_mul(uv, A[:, 0:4 * GH], S_s[ch])
                    nc.vector.tensor_add(S_s[ch][:, 2 * GH:4 * GH],
                                         uv[:, 0:2 * GH], uv[:, 2 * GH:4 * GH])
                    T_ = work.tile([128, 2 * GH], bf16, tag=f"T{ch}")
                    nc.scalar.activation(T_, S_s[ch][:, 2 * GH:4 * GH], AF.Tanh)
                    nc.vector.tensor_mul(h_s[ch], A[:, 4 * GH:6 * GH], T_)

                for tl in range(CHUNK):
                    chain_step(0, tl)
                    chain_step(1, tl)
                    # next chunk's xg matmuls go after both chains' PE phases,
                    # where PE would otherwise stall waiting for the gate tails
                    if chunk + 1 < NCHUNK and tl % 2 == 0 and tl // 2 < NPAIR:
                        xg_evac(xg_nxt, xg_mms(ft_nxt, tl // 2))
                if chunk + 1 < NCHUNK:
                    ft_cur, xg_cur = ft_nxt, xg_nxt
            nc.sync.dma_start(out=hout_d[:, 0:2 * GH], in_=h_s[0])
            nc.sync.dma_start(out=hout_d[:, 2 * GH:4 * GH], in_=h_s[1])
    nc.compile()
    return nc


def _get_bass():
    if "nc" not in _CACHE:
        _CACHE["nc"] = _build_bass()
    return _CACHE["nc"]


def _device_lstm(feat_all, w_ih, bias, w_hh, use_device=True):
    """feat_all: (2B, L, 198) conv features (x1 rows then x2 rows).
    Returns h_final (2B, 256) float32."""
    if not use_device:
        xg = feat_all.reshape(-1, 198) @ w_ih.T + bias
        return _lstm_run(xg.reshape(2 * B, L, 4 * H).astype(np.float32), w_hh)

    from concourse import bass_utils

    # reorder gates [i,f,o,g] for the device packing
    perm = np.r_[0:512, 768:1024, 512:768]
    bf = bfloat16_np()

    whh_r = w_hh[perm, :].copy()                # (1024, 256)
    whh_r[768:1024] *= 2.0                      # fold tanh(g)=2*sig(2g)-1
    whh_host = np.ascontiguousarray(
        whh_r.T.reshape(2, 128, 1024).transpose(1, 0, 2)).astype(bf)

    w_aug = np.zeros((256, 1024), np.float32)
    w_aug[:198] = w_ih[perm, :].T
    w_aug[198] = bias[perm]
    w_aug[:, 768:1024] *= 2.0                   # fold for the g-gate columns
    wih_host = np.ascontiguousarray(
        w_aug.reshape(2, 128, 1024).transpose(1, 0, 2)).astype(bf)

    feat_aug = np.zeros((2 * B, L, 256), np.float32)
    feat_aug[:, :, :198] = feat_all
    feat_aug[:, :, 198] = 1.0

    in_maps = []
    for core in range(NCORES):
        rows = np.concatenate([feat_aug[core * SPC:(core + 1) * SPC],
                               feat_aug[B + core * SPC:B + (core + 1) * SPC]],
                              axis=0)  # (ROWS, L, 256)
        feat_core = np.ascontiguousarray(
            rows.transpose(2, 1, 0).reshape(2, 128, L, ROWS)).astype(bf)
        in_maps.append({
            "feat": feat_core,
            "wih": wih_host,
            "whh": whh_host,
            "ident": np.eye(128, dtype=np.float32).astype(bf),
        })

    nc = _get_bass()
    trace = bool(int(os.environ.get("KERNEL_TRACE", "0")))
    res = bass_utils.run_bass_kernel_spmd(nc, in_maps, core_ids=list(range(NCORES)),
                                          trace=trace)
    if res.exec_time_ns is not None:
        print(f"HW exec time: {res.exec_time_ns} ns")
    elif bool(int(os.environ.get("KERNEL_TIME", "0"))):
        # warm re-run for a wall-clock estimate (compile + first-run overheads
        # amortized away; includes host<->device transfer of in_maps)
        import time
        t0 = time.time()
        res = bass_utils.run_bass_kernel_spmd(nc, in_maps,
                                              core_ids=list(range(NCORES)),
                                              trace=False)
        t1 = time.time()
        print(f"HW exec time: {int((t1 - t0) * 1e9)} ns (warm wall-clock upper bound)")
    h = np.zeros((2 * B, H), np.float32)
    for core in range(NCORES):
        o = np.asarray(res.results[core]["hout"], np.float32)  # (128, 64)
        # cols = [chain(2) x ktile(2) x row(16)]
        hc = o.reshape(128, 2, 2, SPC).transpose(1, 3, 2, 0).reshape(ROWS, 256)
        h[core * SPC:(core + 1) * SPC] = hc[:SPC]
        h[B + core * SPC:B + (core + 1) * SPC] = hc[SPC:]
    return h


def bfloat16_np():
    import ml_dtypes
    return ml_dtypes.bfloat16


def kernel(x1, x2, dtime, conv1_w, conv1_b, conv3_w, conv3_b, conv5_w, conv5_b,
           w_ih_f, w_hh_f, b_ih_f, b_hh_f, w_ih_b, w_hh_b, b_ih_b, b_hh_b,
           fc1_w, fc1_b, fc2_w, fc2_b, use_device=True):
    p = dict(conv1_w=conv1_w, conv1_b=conv1_b, conv3_w=conv3_w, conv3_b=conv3_b,
             conv5_w=conv5_w, conv5_b=conv5_b)
    x1n, x2n, x3n = _preprocess(np.asarray(x1), np.asarray(x2), np.asarray(dtime))
    f1 = _conv_feat(x1n, p)
    f2 = _conv_feat(x2n, p)
    f3 = _conv_feat(x3n, p)

    bias_f = (b_ih_f + b_hh_f).astype(np.float32)
    feat_all = np.concatenate([f1, f2], axis=0)
    h_fwd = _device_lstm(feat_all, w_ih_f.astype(np.float32), bias_f,
                         w_hh_f.astype(np.float32), use_device=use_device)
    hf1, hf2 = h_fwd[:B], h_fwd[B:]

    hb1 = _bwd_cell(f1[:, -1], w_ih_b, w_hh_b, b_ih_b, b_hh_b)
    hb2 = _bwd_cell(f2[:, -1], w_ih_b, w_hh_b, b_ih_b, b_hh_b)

    # x3 branch (L=2): forward 2-step + backward cell, all host
    xg3 = f3.reshape(-1, 198) @ w_ih_f.T.astype(np.float32)
    xg3 = (xg3 + bias_f).reshape(B, 2, 4 * H)
    hf3 = _lstm_run(xg3, w_hh_f.astype(np.float32))
    hb3 = _bwd_cell(f3[:, -1], w_ih_b, w_hh_b, b_ih_b, b_hh_b)

    h1 = np.concatenate([hf1, hb1], axis=-1)
    h2 = np.concatenate([hf2, hb2], axis=-1)
    h3 = np.concatenate([hf3, hb3], axis=-1)
    d = np.concatenate([np.abs(h1 - h2), np.abs(h1 - h3)], axis=-1)
    out = np.maximum(d @ fc1_w.T + fc1_b, 0.0)
    out = _sig(out @ fc2_w.T + fc2_b)
    return out.astype(np.float32)



# revision 9
# speedup vs baseline: 6.7601x; 6.7601x over previous
"""Trainium2 Bass kernel for nn_DCMCLITA (conv + BiLSTM siamese geo model).

Strategy (v2 — minimize per-call overhead, which dominates on this stack):
  - Host (numpy): faithful preprocessing (haversine speed injection, mercator
    normalize), the trivial backward-direction single cells, the tiny x3
    branch (L=2), and the FC head.
  - Device (8 NeuronCores, Bass/Tile): per core one merged 32-row forward
    LSTM chain (16 samples x 2 branches). Conv features are computed ON
    DEVICE from the raw normalized series, so the host->device transfer is
    ~200KB of series data + ~1.1MB of replicated weights per core instead of
    ~9MB of precomputed features. The whole program is a tc.For_i hardware
    loop over 64 chunks of 8 timesteps (~240 instructions total), which
    keeps the per-call BIR->NEFF compile + executable load cheap.

Per-chunk device math:
    xh chunk (6 x (8+4) x 32) -DMA-> SBUF
    conv pre-acts via tap-accumulated matmuls (K=6) into PSUM
    feat tiles: ft0 = [relu(c1)|relu(c3)] (128p), ft1 = [relu(c5)|x|1] (71p)
    xg[m] += wih0[m].T @ ft0 + wih1[m].T @ ft1     (PSUM, 8 m-tiles)
    per t: z[m,t] += whh[k,m].T @ h[k]  (accumulated on top of xg in PSUM)
           A = sigmoid(z_t); tg = 2*A_g - 1 (g pre-scaled x2 host-side)
           u,v = A_i*tg, A_f*c ; c' = u+v ; h = A_o * tanh(c')
"""

import os
import numpy as np

B, L, C, H = 128, 512, 6, 256
NCORES = 8
SPC = B // NCORES          # samples per core
ROWS = 2 * SPC             # 32 rows per core-chain (x1 + x2 branches)
CHUNK = 8                  # timesteps per hw-loop iteration
NCHUNK = L // CHUNK
CC = CHUNK * ROWS          # xg cols per chunk per m-tile (t-major, row-minor)
R_MERC = 6378137.0
R_EARTH = 6371.0

_sig = lambda x: 1.0 / (1.0 + np.exp(-np.clip(x, -80, 80)))


def _conv_feat(x, p):
    # x: (B, L, 6) float32 -> feat (B, L, 198) = [x, relu(c1), relu(c3), relu(c5)]
    outs = [x]
    for K, pad, wk, bk in ((1, 0, 'conv1_w', 'conv1_b'), (3, 1, 'conv3_w', 'conv3_b'),
                           (5, 2, 'conv5_w', 'conv5_b')):
        w, b = p[wk], p[bk]            # (64, 6, K), (64,)
        xp = np.pad(x, ((0, 0), (pad, pad), (0, 0)))
        acc = np.zeros((x.shape[0], x.shape[1], 64), np.float32)
        for j in range(K):
            acc += xp[:, j:j + x.shape[1], :] @ w[:, :, j].T
        outs.append(np.maximum(acc + b, 0.0))
    return np.concatenate(outs, axis=-1).astype(np.float32)


def _merc_x(lon):
    return R_MERC * np.deg2rad(lon)


def _merc_y(lat):
    return R_MERC * np.log(np.tan(np.pi / 4 + np.deg2rad(lat) / 2))


def _preprocess(x1, x2, dtime):
    x1 = x1.astype(np.float32).copy()
    x2 = x2.astype(np.float32).copy()
    lat1, lon1 = x1[:, -1, 0], x1[:, -1, 1]
    lat2, lon2 = x2[:, 0, 0], x2[:, 0, 1]
    la1, lo1, la2, lo2 = map(np.deg2rad, (lat1, lon1, lat2, lon2))
    dlon, dlat = lo2 - lo1, la2 - la1
    a = np.sin(dlat / 2) ** 2 + np.cos(la1) * np.cos(la2) * np.sin(dlon / 2) ** 2
    dist = 2.0 * np.arcsin(np.sqrt(a)) * R_EARTH
    yb = np.sin(dlon) * np.cos(la2)
    xb = np.cos(la1) * np.sin(la2) - np.sin(la1) * np.cos(la2) * np.cos(dlon)
    brg = np.deg2rad((np.degrees(np.arctan2(yb, xb)) + 360.0) % 360.0)
    dt = dtime.reshape(-1).astype(np.float32)
    dt = np.where(dt == 0, np.float32(1.0), dt)
    speeds = dist / dt * 1000.0 / 0.514444
    vx, vy = speeds * np.sin(brg), speeds * np.cos(brg)
    x2[:, 0, 2] = np.where(speeds != 0, speeds, x2[:, 0, 2])
    x2[:, 0, 4] = np.where(vx != 0, vx, x2[:, 0, 4])
    x2[:, 0, 5] = np.where(vy != 0, vy, x2[:, 0, 5])
    x3 = np.concatenate([x1[:, -1:, :], x2[:, 0:1, :]], axis=1)

    a1 = _merc_x(x1[:, :, 1]); b1 = _merc_y(x1[:, :, 0])
    a2 = _merc_x(x2[:, :, 1]); b2 = _merc_y(x2[:, :, 0])
    max_lat = np.maximum(a1.max(1, keepdims=True), a2.max(1, keepdims=True))
    min_lat = np.minimum(a1.min(1, keepdims=True), a2.min(1, keepdims=True))
    max_lon = np.maximum(b1.max(1, keepdims=True), b2.max(1, keepdims=True))
    min_lon = np.minimum(b1.min(1, keepdims=True), b2.min(1, keepdims=True))
    eps = np.float32(1e-8)
    dla = max_lat - min_lat + eps
    dlo = max_lon - min_lon + eps
    x1[:, :, 0] = (a1 - min_lat) / dla; x1[:, :, 1] = (b1 - min_lon) / dlo
    x2[:, :, 0] = (a2 - min_lat) / dla; x2[:, :, 1] = (b2 - min_lon) / dlo
    lat3 = _merc_y(x3[:, :, 0]); lon3 = _merc_x(x3[:, :, 1])
    x3[:, :, 0] = (lat3 - min_lat) / dla; x3[:, :, 1] = (lon3 - min_lon) / dlo
    return x1.astype(np.float32), x2.astype(np.float32), x3.astype(np.float32)


def _lstm_run(xg, w_hh):
    n, T, _ = xg.shape
    h = np.zeros((n, H), np.float32)
    c = np.zeros((n, H), np.float32)
    for t in range(T):
        g = xg[:, t] + h @ w_hh.T
        i, f, gg, o = np.split(g, 4, axis=-1)
        c = _sig(f) * c + _sig(i) * np.tanh(gg)
        h = _sig(o) * np.tanh(c)
    return h


def _bwd_cell(feat_last, w_ih, w_hh, b_ih, b_hh):
    # reference's hb[:, -1] == one LSTM cell applied to the LAST timestep, zero state
    z = feat_last @ w_ih.T + b_ih + b_hh
    i, f, g, o = np.split(z, 4, axis=-1)
    c = _sig(i) * np.tanh(g)
    return _sig(o) * np.tanh(c)


def bfloat16_np():
    import ml_dtypes
    return ml_dtypes.bfloat16


# ---------------------------------------------------------------------------
# Bass device program (built once, cached)
# ---------------------------------------------------------------------------
_CACHE = {}

GPERM = np.r_[0:512, 768:1024, 512:768]  # torch [i,f,g,o] -> m-tile order [i,f,o,g]


def _build_bass(hw_loop=True, nchunk=NCHUNK):
    from contextlib import ExitStack
    import concourse.bass as bass
    import concourse.bacc as bacc
    import concourse.tile as tile
    from concourse import mybir
    from concourse.bass import ds

    nc = bacc.Bacc("TRN2")
    bf16 = mybir.dt.bfloat16
    f32 = mybir.dt.float32
    AF = mybir.ActivationFunctionType

    xh_d = nc.dram_tensor("xh", [6, L + 4, ROWS], bf16, kind="ExternalInput")
    w13_d = nc.dram_tensor("w13", [6, 3, 128], bf16, kind="ExternalInput")
    w5_d = nc.dram_tensor("w5", [6, 5, 71], bf16, kind="ExternalInput")
    cb0_d = nc.dram_tensor("cb0", [128, 1], f32, kind="ExternalInput")
    cb1_d = nc.dram_tensor("cb1", [71, 1], f32, kind="ExternalInput")
    wih0_d = nc.dram_tensor("wih0", [128, 1024], bf16, kind="ExternalInput")
    wih1_d = nc.dram_tensor("wih1", [71, 1024], bf16, kind="ExternalInput")
    whh_d = nc.dram_tensor("whh", [128, 2, 1024], bf16, kind="ExternalInput")
    ident_d = nc.dram_tensor("ident", [128, 128], bf16, kind="ExternalInput")
    hout_d = nc.dram_tensor("hout", [128, 2 * ROWS], bf16, kind="ExternalOutput")

    with tile.TileContext(nc) as tc:
        with ExitStack() as ctx:
            singles = ctx.enter_context(tc.tile_pool(name="singles", bufs=1))
            psums = ctx.enter_context(tc.tile_pool(name="ps", bufs=1, space="PSUM"))

            w13_s = singles.tile([6, 3, 128], bf16)
            nc.sync.dma_start(out=w13_s, in_=w13_d[:])
            w5_s = singles.tile([6, 5, 71], bf16)
            nc.sync.dma_start(out=w5_s, in_=w5_d[:])
            cb0_s = singles.tile([128, 1], f32)
            nc.sync.dma_start(out=cb0_s, in_=cb0_d[:])
            cb1_s = singles.tile([71, 1], f32)
            nc.sync.dma_start(out=cb1_s, in_=cb1_d[:])
            wih0_s = singles.tile([128, 1024], bf16)
            nc.sync.dma_start(out=wih0_s, in_=wih0_d[:])
            wih1_s = singles.tile([71, 1024], bf16)
            nc.sync.dma_start(out=wih1_s, in_=wih1_d[:])
            whh_s = singles.tile([128, 2, 1024], bf16)
            nc.sync.dma_start(out=whh_s, in_=whh_d[:])
            ident_s = singles.tile([128, 128], bf16)
            nc.sync.dma_start(out=ident_s, in_=ident_d[:])

            xh_s = singles.tile([6, CHUNK + 4, ROWS], bf16)
            ft0_s = singles.tile([128, CC], bf16)
            ft1_s = singles.tile([71, CC], bf16)
            xg_s = singles.tile([128, 8, CC], bf16)
            h_s = singles.tile([128, 2 * ROWS], bf16)
            S_s = singles.tile([128, 4 * ROWS], bf16)
            A_s = singles.tile([128, 8 * ROWS], bf16)
            uv_s = singles.tile([128, 4 * ROWS], bf16)
            T_s = singles.tile([128, 2 * ROWS], bf16)
            nc.vector.memset(h_s, 0.0)
            nc.vector.memset(S_s, 0.0)

            pxg = psums.tile([128, CC], f32)        # xg staging (per m-tile)
            z = psums.tile([128, 8 * ROWS], f32)    # per-step gate pre-acts
            pc0 = psums.tile([128, CC], f32)        # conv c1|c3
            pc1 = psums.tile([71, CC], f32)         # conv c5 + x passthrough
            xg4 = xg_s.rearrange("p m (t r) -> p m t r", r=ROWS)

            def chunk_body(ci):
                nc.sync.dma_start(out=xh_s, in_=xh_d[:, ds(ci * CHUNK, CHUNK + 4), :])
                # conv pre-activations (tap-accumulated, K=6 partitions)
                for j in range(3):
                    nc.tensor.matmul(pc0, w13_s[:, j, :],
                                     xh_s[:, j + 1:j + 1 + CHUNK, :],
                                     start=(j == 0), stop=(j == 2))
                for j in range(5):
                    nc.tensor.matmul(pc1, w5_s[:, j, :],
                                     xh_s[:, j:j + CHUNK, :],
                                     start=(j == 0), stop=(j == 4))
                nc.scalar.activation(ft0_s, pc0, AF.Relu, bias=cb0_s)
                nc.scalar.activation(ft1_s[0:64, :], pc1[0:64, :], AF.Relu,
                                     bias=cb1_s[0:64, :])
                nc.scalar.activation(ft1_s[64:71, :], pc1[64:71, :], AF.Identity,
                                     bias=cb1_s[64:71, :])
                # xg[m] = wih0[m].T @ ft0 + wih1[m].T @ ft1, staged to SBUF
                for m in range(8):
                    nc.tensor.matmul(pxg, wih0_s[:, m * 128:(m + 1) * 128],
                                     ft0_s, start=True, stop=False)
                    nc.tensor.matmul(pxg, wih1_s[:, m * 128:(m + 1) * 128],
                                     ft1_s, start=False, stop=True)
                    nc.scalar.copy(xg_s[:, m, :], pxg)
                # recurrence: m-tile order [i0,i1,f0,f1,o0,o1,g0,g1]; g pre-scaled
                # x2 host-side so tanh(g) = 2*sigmoid(2g) - 1. The z group is a
                # single consecutive PE run (identity seed + 16 whh MMs) so the
                # PSUM accumulation stays within one open group.
                for t in range(CHUNK):
                    nc.tensor.matmul(z, ident_s, xg4[:, :, t, :],
                                     start=True, stop=False, skip_group_check=True)
                    for m in range(8):
                        zmt = z[:, m * ROWS:(m + 1) * ROWS]
                        nc.tensor.matmul(zmt, whh_s[:, 0, m * 128:(m + 1) * 128],
                                         h_s[:, 0:ROWS], start=False, stop=False,
                                         skip_group_check=True)
                        nc.tensor.matmul(zmt, whh_s[:, 1, m * 128:(m + 1) * 128],
                                         h_s[:, ROWS:2 * ROWS], start=False,
                                         stop=(m == 7), skip_group_check=True)
                    nc.scalar.activation(A_s, z, AF.Sigmoid)
                    nc.vector.tensor_scalar(S_s[:, 0:2 * ROWS], A_s[:, 6 * ROWS:8 * ROWS],
                                            2.0, -1.0, mybir.AluOpType.mult,
                                            mybir.AluOpType.add)
                    nc.vector.tensor_mul(uv_s, A_s[:, 0:4 * ROWS], S_s)
                    nc.vector.tensor_add(S_s[:, 2 * ROWS:4 * ROWS],
                                         uv_s[:, 0:2 * ROWS], uv_s[:, 2 * ROWS:4 * ROWS])
                    nc.scalar.activation(T_s, S_s[:, 2 * ROWS:4 * ROWS], AF.Tanh)
                    nc.vector.tensor_mul(h_s, A_s[:, 4 * ROWS:6 * ROWS], T_s)

            if hw_loop:
                with tc.For_i(0, nchunk, 1) as ci:
                    chunk_body(ci)
            else:
                for ci in range(nchunk):
                    chunk_body(ci)
            nc.sync.dma_start(out=hout_d[:], in_=h_s)
    nc.compile()
    return nc


def _get_bass():
    if "nc" not in _CACHE:
        _CACHE["nc"] = _build_bass()
    return _CACHE["nc"]


def _pack_weights(w_ih, bias, w_hh, p):
    """Device-layout weight arrays (shared by all cores)."""
    bf = bfloat16_np()
    wp = w_ih[GPERM, :].astype(np.float32)          # (1024, 198)
    wp[768:1024] *= 2.0                             # fold tanh(g)=2*sig(2g)-1
    bp = bias[GPERM].astype(np.float32).copy()
    bp[768:1024] *= 2.0
    wih0 = np.ascontiguousarray(wp[:, 6:134].T).astype(bf)       # [c1|c3] dims
    wih1 = np.concatenate([wp[:, 134:198].T, wp[:, 0:6].T, bp[None, :]],
                          axis=0).astype(bf)                     # [c5|x|bias]

    whh_r = w_hh[GPERM, :].astype(np.float32)
    whh_r[768:1024] *= 2.0
    whh = np.ascontiguousarray(
        whh_r.T.reshape(2, 128, 1024).transpose(1, 0, 2)).astype(bf)

    w1, w3, w5 = p['conv1_w'], p['conv3_w'], p['conv5_w']   # (64, 6, K)
    w13 = np.zeros((6, 3, 128), np.float32)
    for j in range(3):
        if j == 1:
            w13[:, j, 0:64] = w1[:, :, 0].T
        w13[:, j, 64:128] = w3[:, :, j].T
    w5t = np.zeros((6, 5, 71), np.float32)
    for j in range(5):
        w5t[:, j, 0:64] = w5[:, :, j].T
        if j == 2:
            w5t[:, j, 64:70] = np.eye(6)
    cb0 = np.concatenate([p['conv1_b'], p['conv3_b']]).astype(np.float32)[:, None]
    cb1 = np.concatenate([p['conv5_b'], np.zeros(6, np.float32),
                          np.ones(1, np.float32)]).astype(np.float32)[:, None]
    return dict(w13=w13.astype(bf), w5=w5t.astype(bf), cb0=cb0, cb1=cb1,
                wih0=wih0, wih1=wih1, whh=whh)


def _pack_series(x1n, x2n):
    """Per-core [6, L+4, ROWS] bf16 series tensors (2-step zero halo)."""
    bf = bfloat16_np()
    xhs = []
    for core in range(NCORES):
        rows = np.concatenate([x1n[core * SPC:(core + 1) * SPC],
                               x2n[core * SPC:(core + 1) * SPC]], axis=0)  # (32, L, 6)
        xh = np.zeros((6, L + 4, ROWS), np.float32)
        xh[:, 2:L + 2, :] = rows.transpose(2, 1, 0)
        xhs.append(xh.astype(bf))
    return xhs


def _device_lstm(x1n, x2n, w_ih, bias, w_hh, p, use_device=True):
    """Runs the two heavy forward LSTM recurrences on device.
    Returns h_final (2B, 256) float32 (x1 rows then x2 rows)."""
    if not use_device:
        f1 = _conv_feat(x1n, p)
        f2 = _conv_feat(x2n, p)
        feat_all = np.concatenate([f1, f2], axis=0)
        xg = feat_all.reshape(-1, 198) @ w_ih.T + bias
        return _lstm_run(xg.reshape(2 * B, L, 4 * H).astype(np.float32), w_hh)

    from concourse import bass_utils

    wpk = _pack_weights(w_ih, bias, w_hh, p)
    wpk["ident"] = np.eye(128, dtype=np.float32).astype(bfloat16_np())
    xhs = _pack_series(x1n, x2n)
    in_maps = [{"xh": xhs[core], **wpk} for core in range(NCORES)]

    nc = _get_bass()
    trace = bool(int(os.environ.get("KERNEL_TRACE", "0")))
    res = bass_utils.run_bass_kernel_spmd(nc, in_maps, core_ids=list(range(NCORES)),
                                          trace=trace)
    if res.exec_time_ns is not None:
        print(f"HW exec time: {res.exec_time_ns} ns")
    elif bool(int(os.environ.get("KERNEL_TIME", "0"))):
        # warm re-run for a wall-clock estimate (compile + first-run overheads
        # amortized away; includes host<->device transfer of in_maps)
        import time
        t0 = time.time()
        res = bass_utils.run_bass_kernel_spmd(nc, in_maps,
                                              core_ids=list(range(NCORES)),
                                              trace=False)
        t1 = time.time()
        print(f"HW exec time: {int((t1 - t0) * 1e9)} ns (warm wall-clock upper bound)")
    h = np.zeros((2 * B, H), np.float32)
    for core in range(NCORES):
        o = np.asarray(res.results[core]["hout"], np.float32)  # (128, 64)
        # hout[p, k*ROWS + r] = h[row r, hdim 128k+p]
        hc = o.reshape(128, 2, ROWS).transpose(2, 1, 0).reshape(ROWS, 256)
        h[core * SPC:(core + 1) * SPC] = hc[:SPC]
        h[B + core * SPC:B + (core + 1) * SPC] = hc[SPC:]
    return h


def kernel(x1, x2, dtime, conv1_w, conv1_b, conv3_w, conv3_b, conv5_w, conv5_b,
           w_ih_f, w_hh_f, b_ih_f, b_hh_f, w_ih_b, w_hh_b, b_ih_b, b_hh_b,
           fc1_w, fc1_b, fc2_w, fc2_b, use_device=True):
    p = dict(conv1_w=conv1_w.astype(np.float32), conv1_b=conv1_b.astype(np.float32),
             conv3_w=conv3_w.astype(np.float32), conv3_b=conv3_b.astype(np.float32),
             conv5_w=conv5_w.astype(np.float32), conv5_b=conv5_b.astype(np.float32))
    x1n, x2n, x3n = _preprocess(np.asarray(x1), np.asarray(x2), np.asarray(dtime))

    bias_f = (b_ih_f + b_hh_f).astype(np.float32)
    h_fwd = _device_lstm(x1n, x2n, w_ih_f.astype(np.float32), bias_f,
                         w_hh_f.astype(np.float32), p, use_device=use_device)
    hf1, hf2 = h_fwd[:B], h_fwd[B:]

    # backward cells only need the conv features of the LAST timestep
    f1_last = _conv_feat(x1n[:, -5:, :], p)[:, -1]
    f2_last = _conv_feat(x2n[:, -5:, :], p)[:, -1]
    hb1 = _bwd_cell(f1_last, w_ih_b, w_hh_b, b_ih_b, b_hh_b)
    hb2 = _bwd_cell(f2_last, w_ih_b, w_hh_b, b_ih_b, b_hh_b)

    # x3 branch (L=2): forward 2-step + backward cell, all host
    f3 = _conv_feat(x3n, p)
    xg3 = f3.reshape(-1, 198) @ w_ih_f.T.astype(np.float32)
    xg3 = (xg3 + bias_f).reshape(B, 2, 4 * H)
    hf3 = _lstm_run(xg3, w_hh_f.astype(np.float32))
    hb3 = _bwd_cell(f3[:, -1], w_ih_b, w_hh_b, b_ih_b, b_hh_b)

    h1 = np.concatenate([hf1, hb1], axis=-1)
    h2 = np.concatenate([hf2, hb2], axis=-1)
    h3 = np.concatenate([hf3, hb3], axis=-1)
    d = np.concatenate([np.abs(h1 - h2), np.abs(h1 - h3)], axis=-1)
    out = np.maximum(d @ fc1_w.T + fc1_b, 0.0)
    out = _sig(out @ fc2_w.T + fc2_b)
    return out.astype(np.float32)


# revision 15
# speedup vs baseline: 7.5449x; 1.1161x over previous
"""Trainium2 Bass kernel for nn_DCMCLITA (conv + BiLSTM siamese geo model).

Strategy (v2 — minimize per-call overhead, which dominates on this stack):
  - Host (numpy): faithful preprocessing (haversine speed injection, mercator
    normalize), the trivial backward-direction single cells, the tiny x3
    branch (L=2), and the FC head.
  - Device (8 NeuronCores, Bass/Tile): per core one merged 32-row forward
    LSTM chain (16 samples x 2 branches). Conv features are computed ON
    DEVICE from the raw normalized series, so the host->device transfer is
    ~200KB of series data + ~1.1MB of replicated weights per core instead of
    ~9MB of precomputed features. The whole program is a tc.For_i hardware
    loop over 64 chunks of 8 timesteps (~240 instructions total), which
    keeps the per-call BIR->NEFF compile + executable load cheap.

Per-chunk device math:
    xh chunk (6 x (8+4) x 32) -DMA-> SBUF
    conv pre-acts via tap-accumulated matmuls (K=6) into PSUM
    feat tiles: ft0 = [relu(c1)|relu(c3)] (128p), ft1 = [relu(c5)|x|1] (71p)
    xg[m] += wih0[m].T @ ft0 + wih1[m].T @ ft1     (PSUM, 8 m-tiles)
    per t: z[m,t] += whh[k,m].T @ h[k]  (accumulated on top of xg in PSUM)
           A = sigmoid(z_t); tg = 2*A_g - 1 (g pre-scaled x2 host-side)
           u,v = A_i*tg, A_f*c ; c' = u+v ; h = A_o * tanh(c')
"""

import os
import numpy as np

B, L, C, H = 128, 512, 6, 256
NCORES = 8
SPC = B // NCORES          # samples per core
ROWS = 2 * SPC             # 32 rows per core-chain (x1 + x2 branches)
CHUNK = 8                  # timesteps per hw-loop iteration
NCHUNK = L // CHUNK
CC = CHUNK * ROWS          # xg cols per chunk per m-tile (t-major, row-minor)
WALLC = 4992               # packed weight-wall columns (bf16)
R_MERC = 6378137.0
R_EARTH = 6371.0

_sig = lambda x: 1.0 / (1.0 + np.exp(-np.clip(x, -80, 80)))


def _conv_feat(x, p):
    # x: (B, L, 6) float32 -> feat (B, L, 198) = [x, relu(c1), relu(c3), relu(c5)]
    outs = [x]
    for K, pad, wk, bk in ((1, 0, 'conv1_w', 'conv1_b'), (3, 1, 'conv3_w', 'conv3_b'),
                           (5, 2, 'conv5_w', 'conv5_b')):
        w, b = p[wk], p[bk]            # (64, 6, K), (64,)
        xp = np.pad(x, ((0, 0), (pad, pad), (0, 0)))
        acc = np.zeros((x.shape[0], x.shape[1], 64), np.float32)
        for j in range(K):
            acc += xp[:, j:j + x.shape[1], :] @ w[:, :, j].T
        outs.append(np.maximum(acc + b, 0.0))
    return np.concatenate(outs, axis=-1).astype(np.float32)


def _merc_x(lon):
    return R_MERC * np.deg2rad(lon)


def _merc_y(lat):
    return R_MERC * np.log(np.tan(np.pi / 4 + np.deg2rad(lat) / 2))


def _preprocess(x1, x2, dtime):
    x1 = x1.astype(np.float32).copy()
    x2 = x2.astype(np.float32).copy()
    lat1, lon1 = x1[:, -1, 0], x1[:, -1, 1]
    lat2, lon2 = x2[:, 0, 0], x2[:, 0, 1]
    la1, lo1, la2, lo2 = map(np.deg2rad, (lat1, lon1, lat2, lon2))
    dlon, dlat = lo2 - lo1, la2 - la1
    a = np.sin(dlat / 2) ** 2 + np.cos(la1) * np.cos(la2) * np.sin(dlon / 2) ** 2
    dist = 2.0 * np.arcsin(np.sqrt(a)) * R_EARTH
    yb = np.sin(dlon) * np.cos(la2)
    xb = np.cos(la1) * np.sin(la2) - np.sin(la1) * np.cos(la2) * np.cos(dlon)
    brg = np.deg2rad((np.degrees(np.arctan2(yb, xb)) + 360.0) % 360.0)
    dt = dtime.reshape(-1).astype(np.float32)
    dt = np.where(dt == 0, np.float32(1.0), dt)
    speeds = dist / dt * 1000.0 / 0.514444
    vx, vy = speeds * np.sin(brg), speeds * np.cos(brg)
    x2[:, 0, 2] = np.where(speeds != 0, speeds, x2[:, 0, 2])
    x2[:, 0, 4] = np.where(vx != 0, vx, x2[:, 0, 4])
    x2[:, 0, 5] = np.where(vy != 0, vy, x2[:, 0, 5])
    x3 = np.concatenate([x1[:, -1:, :], x2[:, 0:1, :]], axis=1)

    a1 = _merc_x(x1[:, :, 1]); b1 = _merc_y(x1[:, :, 0])
    a2 = _merc_x(x2[:, :, 1]); b2 = _merc_y(x2[:, :, 0])
    max_lat = np.maximum(a1.max(1, keepdims=True), a2.max(1, keepdims=True))
    min_lat = np.minimum(a1.min(1, keepdims=True), a2.min(1, keepdims=True))
    max_lon = np.maximum(b1.max(1, keepdims=True), b2.max(1, keepdims=True))
    min_lon = np.minimum(b1.min(1, keepdims=True), b2.min(1, keepdims=True))
    eps = np.float32(1e-8)
    dla = max_lat - min_lat + eps
    dlo = max_lon - min_lon + eps
    x1[:, :, 0] = (a1 - min_lat) / dla; x1[:, :, 1] = (b1 - min_lon) / dlo
    x2[:, :, 0] = (a2 - min_lat) / dla; x2[:, :, 1] = (b2 - min_lon) / dlo
    lat3 = _merc_y(x3[:, :, 0]); lon3 = _merc_x(x3[:, :, 1])
    x3[:, :, 0] = (lat3 - min_lat) / dla; x3[:, :, 1] = (lon3 - min_lon) / dlo
    return x1.astype(np.float32), x2.astype(np.float32), x3.astype(np.float32)


def _lstm_run(xg, w_hh):
    n, T, _ = xg.shape
    h = np.zeros((n, H), np.float32)
    c = np.zeros((n, H), np.float32)
    for t in range(T):
        g = xg[:, t] + h @ w_hh.T
        i, f, gg, o = np.split(g, 4, axis=-1)
        c = _sig(f) * c + _sig(i) * np.tanh(gg)
        h = _sig(o) * np.tanh(c)
    return h


def _bwd_cell(feat_last, w_ih, w_hh, b_ih, b_hh):
    # reference's hb[:, -1] == one LSTM cell applied to the LAST timestep, zero state
    z = feat_last @ w_ih.T + b_ih + b_hh
    i, f, g, o = np.split(z, 4, axis=-1)
    c = _sig(i) * np.tanh(g)
    return _sig(o) * np.tanh(c)


def bfloat16_np():
    import ml_dtypes
    return ml_dtypes.bfloat16


# ---------------------------------------------------------------------------
# Bass device program (built once, cached)
# ---------------------------------------------------------------------------
_CACHE = {}

GPERM = np.r_[0:512, 768:1024, 512:768]  # torch [i,f,g,o] -> m-tile order [i,f,o,g]


def _build_bass(hw_loop=True, nchunk=NCHUNK):
    from contextlib import ExitStack
    import concourse.bass as bass
    import concourse.bacc as bacc
    import concourse.tile as tile
    from concourse import mybir
    from concourse.bass import ds

    nc = bacc.Bacc("TRN2")
    bf16 = mybir.dt.bfloat16
    f32 = mybir.dt.float32
    AF = mybir.ActivationFunctionType

    xh_d = nc.dram_tensor("xh", [6, L + 4, ROWS], bf16, kind="ExternalInput")
    wall_d = nc.dram_tensor("wall", [128, WALLC], bf16, kind="ExternalInput")
    hout_d = nc.dram_tensor("hout", [128, 2 * ROWS], bf16, kind="ExternalOutput")

    with tile.TileContext(nc) as tc:
        with ExitStack() as ctx:
            singles = ctx.enter_context(tc.tile_pool(name="singles", bufs=1))
            psums = ctx.enter_context(tc.tile_pool(name="ps", bufs=1, space="PSUM"))

            wall_s = singles.tile([128, WALLC], bf16)
            nc.sync.dma_start(out=wall_s, in_=wall_d[:])
            wih0_s = wall_s[:, 0:1024]
            whh_v = lambda k, m: wall_s[:, 1024 + k * 1024 + m * 128:
                                        1024 + k * 1024 + (m + 1) * 128]
            ident_s = wall_s[:, 3072:3200]
            wih1_s = wall_s[0:71, 3200:4224]
            w13_v = lambda j: wall_s[0:6, 4224 + j * 128:4224 + (j + 1) * 128]
            w5_v = lambda j: wall_s[0:6, 4608 + j * 71:4608 + (j + 1) * 71]
            cb0_s = wall_s[:, 4963:4964]
            cb1a_s = wall_s[0:64, 4964:4965]
            cb1b_s = wall_s[64:71, 4964:4965]

            xh_s = singles.tile([6, CHUNK + 4, ROWS], bf16)
            ft0_s = singles.tile([128, CC], bf16)
            ft1_s = singles.tile([71, CC], bf16)
            xg_s = singles.tile([128, 8, CC], bf16)
            h_s = singles.tile([128, 2 * ROWS], bf16)
            S_s = singles.tile([128, 4 * ROWS], bf16)
            A_s = singles.tile([128, 8 * ROWS], bf16)
            uv_s = singles.tile([128, 4 * ROWS], bf16)
            T_s = singles.tile([128, 2 * ROWS], bf16)
            nc.vector.memset(h_s, 0.0)
            nc.vector.memset(S_s, 0.0)

            pxg = psums.tile([128, CC], f32)        # xg staging (per m-tile)
            z = psums.tile([128, 8 * ROWS], f32)    # per-step gate pre-acts
            pc0 = psums.tile([128, CC], f32)        # conv c1|c3
            pc1 = psums.tile([71, CC], f32)         # conv c5 + x passthrough
            xg4 = xg_s.rearrange("p m (t r) -> p m t r", r=ROWS)

            def chunk_body(ci):
                nc.sync.dma_start(out=xh_s, in_=xh_d[:, ds(ci * CHUNK, CHUNK + 4), :])
                # conv pre-activations (tap-accumulated, K=6 partitions)
                for j in range(3):
                    nc.tensor.matmul(pc0, w13_v(j),
                                     xh_s[:, j + 1:j + 1 + CHUNK, :],
                                     start=(j == 0), stop=(j == 2))
                for j in range(5):
                    nc.tensor.matmul(pc1, w5_v(j),
                                     xh_s[:, j:j + CHUNK, :],
                                     start=(j == 0), stop=(j == 4))
                nc.scalar.activation(ft0_s, pc0, AF.Relu, bias=cb0_s)
                nc.scalar.activation(ft1_s[0:64, :], pc1[0:64, :], AF.Relu,
                                     bias=cb1a_s)
                nc.scalar.activation(ft1_s[64:71, :], pc1[64:71, :], AF.Identity,
                                     bias=cb1b_s)
                # xg[m] = wih0[m].T @ ft0 + wih1[m].T @ ft1, staged to SBUF
                for m in range(8):
                    nc.tensor.matmul(pxg, wih0_s[:, m * 128:(m + 1) * 128],
                                     ft0_s, start=True, stop=False)
                    nc.tensor.matmul(pxg, wih1_s[:, m * 128:(m + 1) * 128],
                                     ft1_s, start=False, stop=True)
                    nc.scalar.copy(xg_s[:, m, :], pxg)
                # recurrence: m-tile order [i0,i1,f0,f1,o0,o1,g0,g1]; g pre-scaled
                # x2 host-side so tanh(g) = 2*sigmoid(2g) - 1. The z group is a
                # single consecutive PE run (identity seed + 16 whh MMs) so the
                # PSUM accumulation stays within one open group.
                for t in range(CHUNK):
                    nc.tensor.matmul(z, ident_s, xg4[:, :, t, :],
                                     start=True, stop=False, skip_group_check=True)
                    for m in range(8):
                        zmt = z[:, m * ROWS:(m + 1) * ROWS]
                        nc.tensor.matmul(zmt, whh_v(0, m),
                                         h_s[:, 0:ROWS], start=False, stop=False,
                                         skip_group_check=True)
                        nc.tensor.matmul(zmt, whh_v(1, m),
                                         h_s[:, ROWS:2 * ROWS], start=False,
                                         stop=(m == 7), skip_group_check=True)
                    nc.scalar.activation(A_s, z, AF.Sigmoid)
                    nc.vector.tensor_scalar(S_s[:, 0:2 * ROWS], A_s[:, 6 * ROWS:8 * ROWS],
                                            2.0, -1.0, mybir.AluOpType.mult,
                                            mybir.AluOpType.add)
                    nc.vector.tensor_mul(uv_s, A_s[:, 0:4 * ROWS], S_s)
                    nc.vector.tensor_add(S_s[:, 2 * ROWS:4 * ROWS],
                                         uv_s[:, 0:2 * ROWS], uv_s[:, 2 * ROWS:4 * ROWS])
                    nc.scalar.activation(T_s, S_s[:, 2 * ROWS:4 * ROWS], AF.Tanh)
                    nc.vector.tensor_mul(h_s, A_s[:, 4 * ROWS:6 * ROWS], T_s)

            if hw_loop:
                with tc.For_i(0, nchunk, 1) as ci:
                    chunk_body(ci)
            else:
                for ci in range(nchunk):
                    chunk_body(ci)
            nc.sync.dma_start(out=hout_d[:], in_=h_s)
    nc.compile()
    return nc


def _get_bass():
    if "nc" not in _CACHE:
        _CACHE["nc"] = _build_bass()
    return _CACHE["nc"]


def _pack_weights(w_ih, bias, w_hh, p):
    """Device-layout weight arrays (shared by all cores)."""
    bf = bfloat16_np()
    wp = w_ih[GPERM, :].astype(np.float32)          # (1024, 198)
    wp[768:1024] *= 2.0                             # fold tanh(g)=2*sig(2g)-1
    bp = bias[GPERM].astype(np.float32).copy()
    bp[768:1024] *= 2.0
    wih0 = np.ascontiguousarray(wp[:, 6:134].T).astype(bf)       # [c1|c3] dims
    wih1 = np.concatenate([wp[:, 134:198].T, wp[:, 0:6].T, bp[None, :]],
                          axis=0).astype(bf)                     # [c5|x|bias]

    whh_r = w_hh[GPERM, :].astype(np.float32)
    whh_r[768:1024] *= 2.0
    whh = np.ascontiguousarray(
        whh_r.T.reshape(2, 128, 1024).transpose(1, 0, 2)).astype(bf)

    w1, w3, w5 = p['conv1_w'], p['conv3_w'], p['conv5_w']   # (64, 6, K)
    w13 = np.zeros((6, 3, 128), np.float32)
    for j in range(3):
        if j == 1:
            w13[:, j, 0:64] = w1[:, :, 0].T
        w13[:, j, 64:128] = w3[:, :, j].T
    w5t = np.zeros((6, 5, 71), np.float32)
    for j in range(5):
        w5t[:, j, 0:64] = w5[:, :, j].T
        if j == 2:
            w5t[:, j, 64:70] = np.eye(6)
    cb0 = np.concatenate([p['conv1_b'], p['conv3_b']]).astype(np.float32)
    cb1 = np.concatenate([p['conv5_b'], np.zeros(6, np.float32),
                          np.ones(1, np.float32)]).astype(np.float32)

    wall = np.zeros((128, WALLC), np.float32)
    wall[:, 0:1024] = wih0.astype(np.float32)
    wall[:, 1024:3072] = whh.astype(np.float32).reshape(128, 2048)
    wall[:, 3072:3200] = np.eye(128, dtype=np.float32)
    wall[0:71, 3200:4224] = wih1.astype(np.float32)
    wall[0:6, 4224:4608] = w13.reshape(6, 384)
    wall[0:6, 4608:4963] = w5t.reshape(6, 355)
    wall[:, 4963] = cb0
    wall[0:71, 4964] = cb1
    return wall.astype(bf)


def _pack_series(x1n, x2n):
    """Per-core [6, L+4, ROWS] bf16 series tensors (2-step zero halo)."""
    bf = bfloat16_np()
    xhs = []
    for core in range(NCORES):
        rows = np.concatenate([x1n[core * SPC:(core + 1) * SPC],
                               x2n[core * SPC:(core + 1) * SPC]], axis=0)  # (32, L, 6)
        xh = np.zeros((6, L + 4, ROWS), np.float32)
        xh[:, 2:L + 2, :] = rows.transpose(2, 1, 0)
        xhs.append(xh.astype(bf))
    return xhs


def _device_lstm(x1n, x2n, w_ih, bias, w_hh, p, use_device=True):
    """Runs the two heavy forward LSTM recurrences on device.
    Returns h_final (2B, 256) float32 (x1 rows then x2 rows)."""
    if not use_device:
        f1 = _conv_feat(x1n, p)
        f2 = _conv_feat(x2n, p)
        feat_all = np.concatenate([f1, f2], axis=0)
        xg = feat_all.reshape(-1, 198) @ w_ih.T + bias
        return _lstm_run(xg.reshape(2 * B, L, 4 * H).astype(np.float32), w_hh)

    from concourse import bass_utils

    wall = _pack_weights(w_ih, bias, w_hh, p)
    xhs = _pack_series(x1n, x2n)
    in_maps = [{"xh": xhs[core], "wall": wall} for core in range(NCORES)]

    nc = _get_bass()
    trace = bool(int(os.environ.get("KERNEL_TRACE", "0")))
    res = bass_utils.run_bass_kernel_spmd(nc, in_maps, core_ids=list(range(NCORES)),
                                          trace=trace)
    if res.exec_time_ns is not None:
        print(f"HW exec time: {res.exec_time_ns} ns")
    elif bool(int(os.environ.get("KERNEL_TIME", "0"))):
        # warm re-run for a wall-clock estimate (compile + first-run overheads
        # amortized away; includes host<->device transfer of in_maps)
        import time
        t0 = time.time()
        res = bass_utils.run_bass_kernel_spmd(nc, in_maps,
                                              core_ids=list(range(NCORES)),
                                              trace=False)
        t1 = time.time()
        print(f"HW exec time: {int((t1 - t0) * 1e9)} ns (warm wall-clock upper bound)")
    h = np.zeros((2 * B, H), np.float32)
    for core in range(NCORES):
        o = np.asarray(res.results[core]["hout"], np.float32)  # (128, 64)
        # hout[p, k*ROWS + r] = h[row r, hdim 128k+p]
        hc = o.reshape(128, 2, ROWS).transpose(2, 1, 0).reshape(ROWS, 256)
        h[core * SPC:(core + 1) * SPC] = hc[:SPC]
        h[B + core * SPC:B + (core + 1) * SPC] = hc[SPC:]
    return h


def kernel(x1, x2, dtime, conv1_w, conv1_b, conv3_w, conv3_b, conv5_w, conv5_b,
           w_ih_f, w_hh_f, b_ih_f, b_hh_f, w_ih_b, w_hh_b, b_ih_b, b_hh_b,
           fc1_w, fc1_b, fc2_w, fc2_b, use_device=True):
    p = dict(conv1_w=conv1_w.astype(np.float32), conv1_b=conv1_b.astype(np.float32),
             conv3_w=conv3_w.astype(np.float32), conv3_b=conv3_b.astype(np.float32),
             conv5_w=conv5_w.astype(np.float32), conv5_b=conv5_b.astype(np.float32))
    x1n, x2n, x3n = _preprocess(np.asarray(x1), np.asarray(x2), np.asarray(dtime))

    bias_f = (b_ih_f + b_hh_f).astype(np.float32)
    h_fwd = _device_lstm(x1n, x2n, w_ih_f.astype(np.float32), bias_f,
                         w_hh_f.astype(np.float32), p, use_device=use_device)
    hf1, hf2 = h_fwd[:B], h_fwd[B:]

    # backward cells only need the conv features of the LAST timestep
    f1_last = _conv_feat(x1n[:, -5:, :], p)[:, -1]
    f2_last = _conv_feat(x2n[:, -5:, :], p)[:, -1]
    hb1 = _bwd_cell(f1_last, w_ih_b, w_hh_b, b_ih_b, b_hh_b)
    hb2 = _bwd_cell(f2_last, w_ih_b, w_hh_b, b_ih_b, b_hh_b)

    # x3 branch (L=2): forward 2-step + backward cell, all host
    f3 = _conv_feat(x3n, p)
    xg3 = f3.reshape(-1, 198) @ w_ih_f.T.astype(np.float32)
    xg3 = (xg3 + bias_f).reshape(B, 2, 4 * H)
    hf3 = _lstm_run(xg3, w_hh_f.astype(np.float32))
    hb3 = _bwd_cell(f3[:, -1], w_ih_b, w_hh_b, b_ih_b, b_hh_b)

    h1 = np.concatenate([hf1, hb1], axis=-1)
    h2 = np.concatenate([hf2, hb2], axis=-1)
    h3 = np.concatenate([hf3, hb3], axis=-1)
    d = np.concatenate([np.abs(h1 - h2), np.abs(h1 - h3)], axis=-1)
    out = np.maximum(d @ fc1_w.T + fc1_b, 0.0)
    out = _sig(out @ fc2_w.T + fc2_b)
    return out.astype(np.float32)


# revision 19
# speedup vs baseline: 8.5496x; 1.1332x over previous
"""Trainium2 Bass kernel for nn_DCMCLITA (conv + BiLSTM siamese geo model).

Strategy (v2 — minimize per-call overhead, which dominates on this stack):
  - Host (numpy): faithful preprocessing (haversine speed injection, mercator
    normalize), the trivial backward-direction single cells, the tiny x3
    branch (L=2), and the FC head.
  - Device (8 NeuronCores, Bass/Tile): per core one merged 32-row forward
    LSTM chain (16 samples x 2 branches). Conv features are computed ON
    DEVICE from the raw normalized series, so the host->device transfer is
    ~200KB of series data + ~1.1MB of replicated weights per core instead of
    ~9MB of precomputed features. The whole program is a tc.For_i hardware
    loop over 64 chunks of 8 timesteps (~240 instructions total), which
    keeps the per-call BIR->NEFF compile + executable load cheap.

Per-chunk device math:
    xh chunk (6 x (8+4) x 32) -DMA-> SBUF
    conv pre-acts via tap-accumulated matmuls (K=6) into PSUM
    feat tiles: ft0 = [relu(c1)|relu(c3)] (128p), ft1 = [relu(c5)|x|1] (71p)
    xg[m] += wih0[m].T @ ft0 + wih1[m].T @ ft1     (PSUM, 8 m-tiles)
    per t: z[m,t] += whh[k,m].T @ h[k]  (accumulated on top of xg in PSUM)
           A = sigmoid(z_t); tg = 2*A_g - 1 (g pre-scaled x2 host-side)
           u,v = A_i*tg, A_f*c ; c' = u+v ; h = A_o * tanh(c')
"""

import os
import numpy as np

B, L, C, H = 128, 512, 6, 256
NCORES = 8
SPC = B // NCORES          # samples per core
ROWS = 2 * SPC             # 32 rows per core-chain (x1 + x2 branches)
CHUNK = 8                  # timesteps per hw-loop iteration
NCHUNK = L // CHUNK
CC = CHUNK * ROWS          # xg cols per chunk per m-tile (t-major, row-minor)
WALLC = 4992               # packed weight-wall columns (bf16)
R_MERC = 6378137.0
R_EARTH = 6371.0

_sig = lambda x: 1.0 / (1.0 + np.exp(-np.clip(x, -80, 80)))


def _conv_feat(x, p):
    # x: (B, L, 6) float32 -> feat (B, L, 198) = [x, relu(c1), relu(c3), relu(c5)]
    outs = [x]
    for K, pad, wk, bk in ((1, 0, 'conv1_w', 'conv1_b'), (3, 1, 'conv3_w', 'conv3_b'),
                           (5, 2, 'conv5_w', 'conv5_b')):
        w, b = p[wk], p[bk]            # (64, 6, K), (64,)
        xp = np.pad(x, ((0, 0), (pad, pad), (0, 0)))
        acc = np.zeros((x.shape[0], x.shape[1], 64), np.float32)
        for j in range(K):
            acc += xp[:, j:j + x.shape[1], :] @ w[:, :, j].T
        outs.append(np.maximum(acc + b, 0.0))
    return np.concatenate(outs, axis=-1).astype(np.float32)


def _merc_x(lon):
    return R_MERC * np.deg2rad(lon)


def _merc_y(lat):
    return R_MERC * np.log(np.tan(np.pi / 4 + np.deg2rad(lat) / 2))


def _preprocess(x1, x2, dtime):
    x1 = x1.astype(np.float32).copy()
    x2 = x2.astype(np.float32).copy()
    lat1, lon1 = x1[:, -1, 0], x1[:, -1, 1]
    lat2, lon2 = x2[:, 0, 0], x2[:, 0, 1]
    la1, lo1, la2, lo2 = map(np.deg2rad, (lat1, lon1, lat2, lon2))
    dlon, dlat = lo2 - lo1, la2 - la1
    a = np.sin(dlat / 2) ** 2 + np.cos(la1) * np.cos(la2) * np.sin(dlon / 2) ** 2
    dist = 2.0 * np.arcsin(np.sqrt(a)) * R_EARTH
    yb = np.sin(dlon) * np.cos(la2)
    xb = np.cos(la1) * np.sin(la2) - np.sin(la1) * np.cos(la2) * np.cos(dlon)
    brg = np.deg2rad((np.degrees(np.arctan2(yb, xb)) + 360.0) % 360.0)
    dt = dtime.reshape(-1).astype(np.float32)
    dt = np.where(dt == 0, np.float32(1.0), dt)
    speeds = dist / dt * 1000.0 / 0.514444
    vx, vy = speeds * np.sin(brg), speeds * np.cos(brg)
    x2[:, 0, 2] = np.where(speeds != 0, speeds, x2[:, 0, 2])
    x2[:, 0, 4] = np.where(vx != 0, vx, x2[:, 0, 4])
    x2[:, 0, 5] = np.where(vy != 0, vy, x2[:, 0, 5])
    x3 = np.concatenate([x1[:, -1:, :], x2[:, 0:1, :]], axis=1)

    a1 = _merc_x(x1[:, :, 1]); b1 = _merc_y(x1[:, :, 0])
    a2 = _merc_x(x2[:, :, 1]); b2 = _merc_y(x2[:, :, 0])
    max_lat = np.maximum(a1.max(1, keepdims=True), a2.max(1, keepdims=True))
    min_lat = np.minimum(a1.min(1, keepdims=True), a2.min(1, keepdims=True))
    max_lon = np.maximum(b1.max(1, keepdims=True), b2.max(1, keepdims=True))
    min_lon = np.minimum(b1.min(1, keepdims=True), b2.min(1, keepdims=True))
    eps = np.float32(1e-8)
    dla = max_lat - min_lat + eps
    dlo = max_lon - min_lon + eps
    x1[:, :, 0] = (a1 - min_lat) / dla; x1[:, :, 1] = (b1 - min_lon) / dlo
    x2[:, :, 0] = (a2 - min_lat) / dla; x2[:, :, 1] = (b2 - min_lon) / dlo
    lat3 = _merc_y(x3[:, :, 0]); lon3 = _merc_x(x3[:, :, 1])
    x3[:, :, 0] = (lat3 - min_lat) / dla; x3[:, :, 1] = (lon3 - min_lon) / dlo
    return x1.astype(np.float32), x2.astype(np.float32), x3.astype(np.float32)


def _lstm_run(xg, w_hh):
    n, T, _ = xg.shape
    h = np.zeros((n, H), np.float32)
    c = np.zeros((n, H), np.float32)
    for t in range(T):
        g = xg[:, t] + h @ w_hh.T
        i, f, gg, o = np.split(g, 4, axis=-1)
        c = _sig(f) * c + _sig(i) * np.tanh(gg)
        h = _sig(o) * np.tanh(c)
    return h


def _bwd_cell(feat_last, w_ih, w_hh, b_ih, b_hh):
    # reference's hb[:, -1] == one LSTM cell applied to the LAST timestep, zero state
    z = feat_last @ w_ih.T + b_ih + b_hh
    i, f, g, o = np.split(z, 4, axis=-1)
    c = _sig(i) * np.tanh(g)
    return _sig(o) * np.tanh(c)


def bfloat16_np():
    import ml_dtypes
    return ml_dtypes.bfloat16


# ---------------------------------------------------------------------------
# Bass device program (built once, cached)
# ---------------------------------------------------------------------------
_CACHE = {}

GPERM = np.r_[0:512, 768:1024, 512:768]  # torch [i,f,g,o] -> m-tile order [i,f,o,g]


def _build_bass(hw_loop=True, nchunk=NCHUNK, wall_np=None):
    from contextlib import ExitStack
    import concourse.bass as bass
    import concourse.bacc as bacc
    import concourse.tile as tile
    from concourse import mybir
    from concourse.bass import ds

    nc = bacc.Bacc("TRN2")
    bf16 = mybir.dt.bfloat16
    f32 = mybir.dt.float32
    AF = mybir.ActivationFunctionType

    xh_d = nc.dram_tensor("xh", [6, L + 4, ROWS], bf16, kind="ExternalInput")
    if wall_np is not None:
        wall_d = nc.inline_tensor(wall_np, name="wall")
    else:
        wall_d = nc.dram_tensor("wall", [128, WALLC], bf16, kind="ExternalInput")
    hout_d = nc.dram_tensor("hout", [128, 2 * ROWS], bf16, kind="ExternalOutput")

    with tile.TileContext(nc) as tc:
        with ExitStack() as ctx:
            singles = ctx.enter_context(tc.tile_pool(name="singles", bufs=1))
            psums = ctx.enter_context(tc.tile_pool(name="ps", bufs=1, space="PSUM"))

            wall_s = singles.tile([128, WALLC], bf16)
            nc.sync.dma_start(out=wall_s, in_=wall_d[:])
            wih0_s = wall_s[:, 0:1024]
            whh_v = lambda k, m: wall_s[:, 1024 + k * 1024 + m * 128:
                                        1024 + k * 1024 + (m + 1) * 128]
            ident_s = wall_s[:, 3072:3200]
            wih1_s = wall_s[0:71, 3200:4224]
            w13_v = lambda j: wall_s[0:6, 4224 + j * 128:4224 + (j + 1) * 128]
            w5_v = lambda j: wall_s[0:6, 4608 + j * 71:4608 + (j + 1) * 71]
            cb0_s = wall_s[:, 4963:4964]
            cb1a_s = wall_s[0:64, 4964:4965]
            cb1b_s = wall_s[64:71, 4964:4965]

            xh_s = singles.tile([6, CHUNK + 4, ROWS], bf16)
            ft0_s = singles.tile([128, CC], bf16)
            ft1_s = singles.tile([71, CC], bf16)
            xg_s = singles.tile([128, 8, CC], bf16)
            h_s = singles.tile([128, 2 * ROWS], bf16)
            S_s = singles.tile([128, 4 * ROWS], bf16)
            A_s = singles.tile([128, 8 * ROWS], bf16)
            uv_s = singles.tile([128, 4 * ROWS], bf16)
            T_s = singles.tile([128, 2 * ROWS], bf16)
            nc.vector.memset(h_s, 0.0)
            nc.vector.memset(S_s, 0.0)

            pxg = psums.tile([128, CC], f32)        # xg staging (per m-tile)
            z = psums.tile([128, 8 * ROWS], f32)    # per-step gate pre-acts
            pc0 = psums.tile([128, CC], f32)        # conv c1|c3
            pc1 = psums.tile([71, CC], f32)         # conv c5 + x passthrough
            xg4 = xg_s.rearrange("p m (t r) -> p m t r", r=ROWS)

            def chunk_body(ci):
                nc.sync.dma_start(out=xh_s, in_=xh_d[:, ds(ci * CHUNK, CHUNK + 4), :])
                # conv pre-activations (tap-accumulated, K=6 partitions)
                for j in range(3):
                    nc.tensor.matmul(pc0, w13_v(j),
                                     xh_s[:, j + 1:j + 1 + CHUNK, :],
                                     start=(j == 0), stop=(j == 2))
                for j in range(5):
                    nc.tensor.matmul(pc1, w5_v(j),
                                     xh_s[:, j:j + CHUNK, :],
                                     start=(j == 0), stop=(j == 4))
                nc.scalar.activation(ft0_s, pc0, AF.Relu, bias=cb0_s)
                nc.scalar.activation(ft1_s[0:64, :], pc1[0:64, :], AF.Relu,
                                     bias=cb1a_s)
                nc.scalar.activation(ft1_s[64:71, :], pc1[64:71, :], AF.Identity,
                                     bias=cb1b_s)
                # xg[m] = wih0[m].T @ ft0 + wih1[m].T @ ft1, staged to SBUF
                for m in range(8):
                    nc.tensor.matmul(pxg, wih0_s[:, m * 128:(m + 1) * 128],
                                     ft0_s, start=True, stop=False)
                    nc.tensor.matmul(pxg, wih1_s[:, m * 128:(m + 1) * 128],
                                     ft1_s, start=False, stop=True)
                    nc.scalar.copy(xg_s[:, m, :], pxg)
                # recurrence: m-tile order [i0,i1,f0,f1,o0,o1,g0,g1]; g pre-scaled
                # x2 host-side so tanh(g) = 2*sigmoid(2g) - 1. The z group is a
                # single consecutive PE run (identity seed + 16 whh MMs) so the
                # PSUM accumulation stays within one open group.
                for t in range(CHUNK):
                    nc.tensor.matmul(z, ident_s, xg4[:, :, t, :],
                                     start=True, stop=False, skip_group_check=True)
                    for m in range(8):
                        zmt = z[:, m * ROWS:(m + 1) * ROWS]
                        nc.tensor.matmul(zmt, whh_v(0, m),
                                         h_s[:, 0:ROWS], start=False, stop=False,
                                         skip_group_check=True)
                        nc.tensor.matmul(zmt, whh_v(1, m),
                                         h_s[:, ROWS:2 * ROWS], start=False,
                                         stop=(m == 7), skip_group_check=True)
                    nc.scalar.activation(A_s, z, AF.Sigmoid)
                    nc.vector.tensor_scalar(S_s[:, 0:2 * ROWS], A_s[:, 6 * ROWS:8 * ROWS],
                                            2.0, -1.0, mybir.AluOpType.mult,
                                            mybir.AluOpType.add)
                    nc.vector.tensor_mul(uv_s, A_s[:, 0:4 * ROWS], S_s)
                    nc.vector.tensor_add(S_s[:, 2 * ROWS:4 * ROWS],
                                         uv_s[:, 0:2 * ROWS], uv_s[:, 2 * ROWS:4 * ROWS])
                    nc.scalar.activation(T_s, S_s[:, 2 * ROWS:4 * ROWS], AF.Tanh)
                    nc.vector.tensor_mul(h_s, A_s[:, 4 * ROWS:6 * ROWS], T_s)

            if hw_loop:
                with tc.For_i(0, nchunk, 1) as ci:
                    chunk_body(ci)
            else:
                for ci in range(nchunk):
                    chunk_body(ci)
            nc.sync.dma_start(out=hout_d[:], in_=h_s)
    nc.compile()
    return nc


def _get_bass(wall_np=None):
    if "nc" not in _CACHE:
        _CACHE["nc"] = _build_bass(wall_np=wall_np)
    return _CACHE["nc"]


def _pack_weights(w_ih, bias, w_hh, p):
    """Device-layout weight arrays (shared by all cores)."""
    bf = bfloat16_np()
    wp = w_ih[GPERM, :].astype(np.float32)          # (1024, 198)
    wp[768:1024] *= 2.0                             # fold tanh(g)=2*sig(2g)-1
    bp = bias[GPERM].astype(np.float32).copy()
    bp[768:1024] *= 2.0
    wih0 = np.ascontiguousarray(wp[:, 6:134].T).astype(bf)       # [c1|c3] dims
    wih1 = np.concatenate([wp[:, 134:198].T, wp[:, 0:6].T, bp[None, :]],
                          axis=0).astype(bf)                     # [c5|x|bias]

    whh_r = w_hh[GPERM, :].astype(np.float32)
    whh_r[768:1024] *= 2.0
    whh = np.ascontiguousarray(
        whh_r.T.reshape(2, 128, 1024).transpose(1, 0, 2)).astype(bf)

    w1, w3, w5 = p['conv1_w'], p['conv3_w'], p['conv5_w']   # (64, 6, K)
    w13 = np.zeros((6, 3, 128), np.float32)
    for j in range(3):
        if j == 1:
            w13[:, j, 0:64] = w1[:, :, 0].T
        w13[:, j, 64:128] = w3[:, :, j].T
    w5t = np.zeros((6, 5, 71), np.float32)
    for j in range(5):
        w5t[:, j, 0:64] = w5[:, :, j].T
        if j == 2:
            w5t[:, j, 64:70] = np.eye(6)
    cb0 = np.concatenate([p['conv1_b'], p['conv3_b']]).astype(np.float32)
    cb1 = np.concatenate([p['conv5_b'], np.zeros(6, np.float32),
                          np.ones(1, np.float32)]).astype(np.float32)

    wall = np.zeros((128, WALLC), np.float32)
    wall[:, 0:1024] = wih0.astype(np.float32)
    wall[:, 1024:3072] = whh.astype(np.float32).reshape(128, 2048)
    wall[:, 3072:3200] = np.eye(128, dtype=np.float32)
    wall[0:71, 3200:4224] = wih1.astype(np.float32)
    wall[0:6, 4224:4608] = w13.reshape(6, 384)
    wall[0:6, 4608:4963] = w5t.reshape(6, 355)
    wall[:, 4963] = cb0
    wall[0:71, 4964] = cb1
    return wall.astype(bf)


def _pack_series(x1n, x2n):
    """Per-core [6, L+4, ROWS] bf16 series tensors (2-step zero halo)."""
    bf = bfloat16_np()
    xhs = []
    for core in range(NCORES):
        rows = np.concatenate([x1n[core * SPC:(core + 1) * SPC],
                               x2n[core * SPC:(core + 1) * SPC]], axis=0)  # (32, L, 6)
        xh = np.zeros((6, L + 4, ROWS), np.float32)
        xh[:, 2:L + 2, :] = rows.transpose(2, 1, 0)
        xhs.append(xh.astype(bf))
    return xhs


def _device_lstm(x1n, x2n, w_ih, bias, w_hh, p, use_device=True):
    """Runs the two heavy forward LSTM recurrences on device.
    Returns h_final (2B, 256) float32 (x1 rows then x2 rows)."""
    if not use_device:
        f1 = _conv_feat(x1n, p)
        f2 = _conv_feat(x2n, p)
        feat_all = np.concatenate([f1, f2], axis=0)
        xg = feat_all.reshape(-1, 198) @ w_ih.T + bias
        return _lstm_run(xg.reshape(2 * B, L, 4 * H).astype(np.float32), w_hh)

    from concourse import bass_utils

    wall = _pack_weights(w_ih, bias, w_hh, p)
    xhs = _pack_series(x1n, x2n)
    in_maps = [{"xh": xhs[core]} for core in range(NCORES)]

    nc = _get_bass(wall_np=wall)
    trace = bool(int(os.environ.get("KERNEL_TRACE", "0")))
    res = bass_utils.run_bass_kernel_spmd(nc, in_maps, core_ids=list(range(NCORES)),
                                          trace=trace)
    if res.exec_time_ns is not None:
        print(f"HW exec time: {res.exec_time_ns} ns")
    elif bool(int(os.environ.get("KERNEL_TIME", "0"))):
        # warm re-run for a wall-clock estimate (compile + first-run overheads
        # amortized away; includes host<->device transfer of in_maps)
        import time
        t0 = time.time()
        res = bass_utils.run_bass_kernel_spmd(nc, in_maps,
                                              core_ids=list(range(NCORES)),
                                              trace=False)
        t1 = time.time()
        print(f"HW exec time: {int((t1 - t0) * 1e9)} ns (warm wall-clock upper bound)")
    h = np.zeros((2 * B, H), np.float32)
    for core in range(NCORES):
        o = np.asarray(res.results[core]["hout"], np.float32)  # (128, 64)
        # hout[p, k*ROWS + r] = h[row r, hdim 128k+p]
        hc = o.reshape(128, 2, ROWS).transpose(2, 1, 0).reshape(ROWS, 256)
        h[core * SPC:(core + 1) * SPC] = hc[:SPC]
        h[B + core * SPC:B + (core + 1) * SPC] = hc[SPC:]
    return h


def kernel(x1, x2, dtime, conv1_w, conv1_b, conv3_w, conv3_b, conv5_w, conv5_b,
           w_ih_f, w_hh_f, b_ih_f, b_hh_f, w_ih_b, w_hh_b, b_ih_b, b_hh_b,
           fc1_w, fc1_b, fc2_w, fc2_b, use_device=True):
    p = dict(conv1_w=conv1_w.astype(np.float32), conv1_b=conv1_b.astype(np.float32),
             conv3_w=conv3_w.astype(np.float32), conv3_b=conv3_b.astype(np.float32),
             conv5_w=conv5_w.astype(np.float32), conv5_b=conv5_b.astype(np.float32))
    x1n, x2n, x3n = _preprocess(np.asarray(x1), np.asarray(x2), np.asarray(dtime))

    bias_f = (b_ih_f + b_hh_f).astype(np.float32)
    h_fwd = _device_lstm(x1n, x2n, w_ih_f.astype(np.float32), bias_f,
                         w_hh_f.astype(np.float32), p, use_device=use_device)
    hf1, hf2 = h_fwd[:B], h_fwd[B:]

    # backward cells only need the conv features of the LAST timestep
    f1_last = _conv_feat(x1n[:, -5:, :], p)[:, -1]
    f2_last = _conv_feat(x2n[:, -5:, :], p)[:, -1]
    hb1 = _bwd_cell(f1_last, w_ih_b, w_hh_b, b_ih_b, b_hh_b)
    hb2 = _bwd_cell(f2_last, w_ih_b, w_hh_b, b_ih_b, b_hh_b)

    # x3 branch (L=2): forward 2-step + backward cell, all host
    f3 = _conv_feat(x3n, p)
    xg3 = f3.reshape(-1, 198) @ w_ih_f.T.astype(np.float32)
    xg3 = (xg3 + bias_f).reshape(B, 2, 4 * H)
    hf3 = _lstm_run(xg3, w_hh_f.astype(np.float32))
    hb3 = _bwd_cell(f3[:, -1], w_ih_b, w_hh_b, b_ih_b, b_hh_b)

    h1 = np.concatenate([hf1, hb1], axis=-1)
    h2 = np.concatenate([hf2, hb2], axis=-1)
    h3 = np.concatenate([hf3, hb3], axis=-1)
    d = np.concatenate([np.abs(h1 - h2), np.abs(h1 - h3)], axis=-1)
    out = np.maximum(d @ fc1_w.T + fc1_b, 0.0)
    out = _sig(out @ fc2_w.T + fc2_b)
    return out.astype(np.float32)
